# revision 59
# baseline (speedup 1.0000x reference)
"""Fused transformer block v3 (RMSNorm + qk-norm attention + MLP), TRN2, 8 cores.

Sharding: 8 cores = (4 batches) x (2 query-halves). Each core's rows are
rotated so its query half is rows 0..1023; K/V cover the full sequence.

v3 highlights (each driven by a perfetto finding):
 - fp8e4 DoubleRow QKV projections: x^T and the (x32 pre-scaled) weights ship
   as fp8; each matmul contracts two 128-row model-dim tiles -> ~2x PE rate.
   The x32 washes out of Q/K through the qk-rmsnorm; V folds 1/32 into the
   host-precomputed ln1 1/rms scale (rs_t) applied at PSUM evacuation.
 - K is never normalized on chip: its per-(key,head) 1/rms lands as the
   per-partition scale AP of the softmax exp (rsK), computed by one deferred
   Sqrt+reciprocal over all 16 key tiles' stats.
 - The attention phase was ACT-bound (25.2M softmax exp elems = 164us at
   1/cycle/lane): odd heads' exp runs on DVE via a 16-bit Schraudolph
   (i16 = C1*l + C2 builds bf16 prob bits directly; softmax renormalizes the
   +-3.5% piecewise-linear error away). Even heads' exp stays on ACT but
   emits fp8 probs (exp bias -5ln2 keeps e^8 under e4m3's max; the 1/32
   cancels through the softmax divide), feeding fp8 DoubleRow AV matmuls
   that contract two key tiles each.
 - Lag-1 software pipelining in the attention loop: AV for tile t-1 issues
   after the logits for tile t so exp latency hides outside the in-order PE
   queue; denominator reciprocals batched per head pair and broadcast with a
   bf16 ones-block matmul.
 - Phase D (attention-out proj + ln2) runs transposed in 4x256 chunks so
   each chunk's stats chain hides under the next chunk's matmuls.
 - MLP1 runs fp8 DoubleRow (wi pre-scaled x32, undone inside gelu's
   pre-affine; x2h quantized to fp8 at the ln2 multiply). MLP2 stays bf16 --
   a second fp8 stage would push the deterministic rel-err (1.34e-2) too
   close to the 2e-2 gate. Both MLP stages run transposed; the host
   transposes the [D, SQ] fp32 output back during the gather.
 - DRAM round trips for K^T/Q^T batch 4 tiles per SWDGE call; transposes
   split across the two HWDGE rings (sync + scalar).
"""

import numpy as np
from contextlib import ExitStack

import concourse.bass as bass
import concourse.tile as tile
from concourse import bacc, mybir
from concourse.bass_utils import run_bass_kernel_spmd

F32 = mybir.dt.float32
BF16 = mybir.dt.bfloat16
AF = mybir.ActivationFunctionType
OP = mybir.AluOpType

B, S, D, H, HD, MLP = 4, 2048, 768, 12, 64, 3072
SQ = S // 2            # query rows per core
NT_S = S // 128        # 16 sequence tiles
NT_Q = SQ // 128       # 8 query tiles
NT_D = D // 128        # 6 model-dim tiles
NT_M = MLP // 128      # 24 mlp-dim tiles
EPS = 1e-6
VW = HD + 1            # V width incl. ones column
VPAD = 128 - VW        # garbage pad so every AV stationary slice is 128 wide
# Schraudolph fast-exp in bf16 bit-space: i16 = trunc(C1*x + C2) builds the
# bf16 bit pattern of ~exp(x) directly (exp goes in the high bits, linear
# mantissa interp in the low 7). C2 shift tuned for min max-rel-error
# (+-3.5%); softmax renormalizes so the end-to-end delta is ~1e-4.
# Logits are bounded (|l| <= 8: q,k are rms-normalized and q carries 1/8).
FEXP_C1 = float(128.0 / np.log(2.0))
FEXP_C2 = float(127 * 128 - 0.043677 * 128)


def _chunks(n, c=512):
    out, ofs = [], 0
    while ofs < n:
        m = min(c, n - ofs)
        out.append((ofs, m))
        ofs += m
    return out


def build_nc(sim_compat=False):
    nc = bacc.Bacc("TRN2", target_bir_lowering=False, debug=False, num_devices=8)

    F8 = mybir.dt.float8e4
    # lat8: latents^T in fp8 for the QKV projections (DoubleRow) and the rs
    # stats; latb: bf16 residual half (the residual add needs full precision).
    # Weights come in pre-scaled by 32 so their ~0.02-sd values sit in e4m3's
    # normal range; the 1/32 washes out of Q/K via the qk-rmsnorm and is
    # folded into rs_t for V.
    lat8 = nc.dram_tensor("lat8", [D, S], F8, kind="ExternalInput").ap()
    latb = nc.dram_tensor("latb", [D, SQ], BF16, kind="ExternalInput").ap()
    # Host-precomputed ln1 1/rms per token (incl. the 1/32 fp8 weight
    # compensation), laid out [token%128, token//128].
    rst = nc.dram_tensor("rst", [128, NT_S], F32, kind="ExternalInput").ap()
    wq = nc.dram_tensor("wq", [D, D], F8, kind="ExternalInput").ap()
    wk = nc.dram_tensor("wk", [D, D], F8, kind="ExternalInput").ap()
    wv = nc.dram_tensor("wv", [D, D], F8, kind="ExternalInput").ap()
    wo = nc.dram_tensor("wo", [D, D], BF16, kind="ExternalInput").ap()
    wi = nc.dram_tensor("wi", [D, MLP], F8, kind="ExternalInput").ap()
    wom = nc.dram_tensor("wom", [MLP, D], BF16, kind="ExternalInput").ap()
    out = nc.dram_tensor("out", [D, SQ], F32, kind="ExternalOutput").ap()

    with tile.TileContext(nc) as tc, ExitStack() as top:
        def ptile(pool, shape, dtype, name):
            return pool.tile(shape, dtype, name=name, tag=name)

        p_const = top.enter_context(tc.tile_pool(name="p_const", bufs=1))
        p_oT = tc.alloc_tile_pool(name="p_oT", bufs=1)
        p_wo = tc.alloc_tile_pool(name="p_wo", bufs=1)
        p_xT = tc.alloc_tile_pool(name="p_xT", bufs=1)
        p_att = tc.alloc_tile_pool(name="p_att", bufs=1)

        # ---- persistent tiles ----
        ones_col = ptile(p_const, [128, 1], BF16, name="ones_col")
        eps_t = ptile(p_const, [128, 1], F32, name="eps_t")
        bc_pair = ptile(p_const, [128, 128], BF16, name="bc_pair")
        bc_row0 = ptile(p_const, [128, 128], BF16, name="bc_row0")
        rs_t = ptile(p_const, [128, NT_S], F32, name="rs_t")
        rs2_pad = ptile(p_const, [128, SQ], BF16, name="rs2_pad")

        x8 = [ptile(p_xT, [128, 2, S], mybir.dt.float8e4, name=f"x8_{j}")
              for j in range(NT_D // 2)]
        # V split by head parity: even heads go to an fp8 copy consumed by
        # DoubleRow AV matmuls (two key-tiles contracted per MM); odd heads
        # stay bf16. Each even head gets a full 128-wide slot (65 = V+ones,
        # rest zero) so the DR stationary window never crosses a boundary.
        va8 = ptile(p_att, [128, NT_S, 6, 128], mybir.dt.float8e4,
                    name="va8")
        va16 = ptile(p_att, [128, NT_S * 6 * VW + VPAD], BF16, name="va16")
        KT = [ptile(p_att, [128, S], BF16, name=f"KT{d}") for d in range(NT_D)]
        QTz = [[ptile(p_att, [128, SQ], BF16, name=f"QTz{d}_{e}")
                for e in range(2)] for d in range(NT_D)]
        rT_pad = ptile(p_att, [128, SQ], BF16, name="rT_pad")
        dn_p = ptile(p_att, [VW, SQ], F32, name="dn_p")
        # Per-(key, head) qk-norm reciprocals: K is stored RAW; its 1/rms is
        # applied inside the softmax exp as a per-partition scale AP (ACT) /
        # AP-scalar (DVE fast-exp). rsKc1 = rsK * FEXP_C1 for the DVE path.
        rsK = ptile(p_att, [128, NT_S, H], F32, name="rsK")
        rsKc1 = ptile(p_att, [128, NT_S, H], F32, name="rsKc1")
        oT = ptile(p_oT, [128, NT_D * SQ], BF16, name="oT")
        wo_sb = [ptile(p_wo, [128, D], BF16, name=f"wo_sb{d}") for d in range(NT_D)]

        eps1k_t = ptile(p_const, [128, 1], F32, name="eps1k_t")
        mexp5_t = ptile(p_const, [128, 1], F32, name="mexp5_t")
        nc.vector.memset(ones_col[:], 1.0)
        nc.vector.memset(eps_t[:], EPS)
        nc.vector.memset(eps1k_t[:], EPS * 1024.0)
        nc.vector.memset(mexp5_t[:], -5.0 * float(np.log(2.0)))
        nc.vector.memset(bc_pair[:], 0.0)
        nc.vector.memset(bc_pair[0:1, 0:64], 1.0)
        nc.vector.memset(bc_pair[64:65, 64:128], 1.0)
        nc.vector.memset(bc_row0[:], 0.0)
        nc.vector.memset(bc_row0[0:1, :], 1.0)
        nc.vector.memset(rT_pad[:], 0.0)
        nc.vector.memset(rs2_pad[:], 0.0)
        nc.vector.memset(dn_p[:], 1.0)
        nc.sync.dma_start(rs_t[:], rst[:])
        v8view = va8
        v16view = va16[:, 0:NT_S * 6 * VW].rearrange(
            "p (s h k) -> p s h k", s=NT_S, h=6)
        nc.vector.memset(va8[:, :, :, HD:VW], 1.0)
        nc.vector.memset(va8[:, :, :, VW:], 0.0)
        nc.vector.memset(v16view[:, :, :, HD:VW], 1.0)
        nc.vector.memset(va16[:, NT_S * 6 * VW:], 0.0)
        for d in range(NT_D):
            for e in range(2):
                nc.vector.memset(QTz[d][e][:], 0.0)

        dram = top.enter_context(tc.tile_pool(name="dram", bufs=1, space="DRAM"))
        kh_d = dram.tile([S, D], BF16, name="kh_d")
        qh_d = dram.tile([SQ, D], BF16, name="qh_d")


        # =============== Phase B: rs + Q/K/V projections + qk-norm ========
        p_qtf = tc.alloc_tile_pool(name="p_qtf", bufs=1)
        QTf = [ptile(p_qtf, [128, SQ], BF16, name=f"QTf{d}") for d in range(NT_D)]
        # allocate ALL p_qtf tiles up front -- tiles requested after other
        # pools exist can land on overlapping SBUF regions
        ssK = ptile(p_qtf, [128, NT_S * H], F32, name="ssK")
        srtK_t = ptile(p_qtf, [128, NT_S * H], F32, name="srtK_t")
        with ExitStack() as ctx:
            F8 = mybir.dt.float8e4
            NJ = NT_D // 2
            wp = ctx.enter_context(tc.tile_pool(name="b_w", bufs=1))
            wq_sb = [wp.tile([128, 2, D], F8, name=f"wq_sb{j}") for j in range(NJ)]
            wk_sb = [wp.tile([128, 2, D], F8, name=f"wk_sb{j}") for j in range(NJ)]
            wv_sb = [wp.tile([128, 2, D], F8, name=f"wv_sb{j}") for j in range(NJ)]

            def xs(d):
                # [128, S] fp8 view of model-dim tile d of lat^T
                return x8[d // 2][:, d % 2]

            # Batched loads: one DMA per (pair, wide chunk) -- per-call issue
            # cost on the sync queue was a phase-B serializer at 42 calls.
            for j in range(NJ):
                nc.sync.dma_start(x8[j][:, :, 0:512],
                                  lat8[2 * j * 128:(2 * j + 2) * 128,
                                       0:512].rearrange(
                                      "(e p) s -> p e s", e=2))
                nc.sync.dma_start(wk_sb[j][:],
                                  wk[2 * j * 128:(2 * j + 2) * 128,
                                     :].rearrange("(e p) s -> p e s", e=2))
            for ofs, n in _chunks(S):
                if ofs == 0:
                    continue
                for j in range(NJ):
                    nc.sync.dma_start(x8[j][:, :, ofs:ofs + n],
                                      lat8[2 * j * 128:(2 * j + 2) * 128,
                                           ofs:ofs + n].rearrange(
                                          "(e p) s -> p e s", e=2))
            for j in range(NJ):
                nc.sync.dma_start(wq_sb[j][:],
                                  wq[2 * j * 128:(2 * j + 2) * 128,
                                     :].rearrange("(e p) s -> p e s", e=2))
            for j in range(NJ):
                nc.sync.dma_start(wv_sb[j][:],
                                  wv[2 * j * 128:(2 * j + 2) * 128,
                                     :].rearrange("(e p) s -> p e s", e=2))

            ps = ctx.enter_context(tc.tile_pool(name="b_ps", bufs=3, space="PSUM"))
            scr = ctx.enter_context(tc.tile_pool(name="b_scr", bufs=4))
            natp = ctx.enter_context(tc.tile_pool(name="b_nat", bufs=4))
            st_p = ctx.enter_context(tc.tile_pool(name="b_stats", bufs=8))

            def proj(t, w_sb):
                # fp8 DoubleRow: each matmul contracts two 128-row d-tiles
                # (stationary = x^T pair, moving = weight pair) -> 2x PE rate.
                p = ps.tile([128, D], F32, name="p_proj")
                for j in range(NJ):
                    lhsT = x8[j][:, :, t * 128:(t + 1) * 128]
                    for ofs, n in _chunks(D):
                        nc.tensor.matmul(
                            p[:, ofs:ofs + n], lhsT, w_sb[j][:, :, ofs:ofs + n],
                            start=(j == 0), stop=(j == NJ - 1),
                            perf_mode=mybir.MatmulPerfMode.DoubleRow)
                return p

            natg = [None]

            def dma_batch(dst_dram, t):
                if t % 4 == 3:
                    # One batched DRAM write per 4 tiles: the per-call SWDGE
                    # issue cost serialized the gpsimd queue at 40 calls.
                    dst = dst_dram[(t - 3) * 128:(t + 1) * 128, :].rearrange(
                        "(f p) d -> p f d", f=4)
                    nc.gpsimd.dma_start(dst, natg[0][:])

            def knorm(p, t):
                # K path: store RAW k (ACT Copy straight into the DMA group
                # tile); only the per-head sum-of-squares is produced here --
                # the Rsqrt for ALL K tiles runs as ONE deferred ACT op at the
                # end of the loop (an inline Sqrt head-of-line-blocked the ACT
                # FIFO behind the DVE reduce and starved the PE of PSUM slots).
                if t % 4 == 0:
                    natg[0] = natp.tile([128, 4, D], BF16, name="nat_b",
                                        tag="nat_b")
                praw = natg[0][:, t % 4]
                nc.scalar.activation(praw, p[:], AF.Copy)
                sqv = scr.tile([128, D], BF16, name="sq_b", tag="sq_b")
                sq_eng = nc.gpsimd if t % 4 != 3 else nc.vector
                sq_eng.tensor_tensor(out=sqv[:], in0=praw, in1=praw,
                                     op=OP.mult)
                nc.vector.tensor_reduce(
                    ssK[:, t * H:(t + 1) * H],
                    sqv[:].rearrange("p (h k) -> p h k", h=H),
                    axis=mybir.AxisListType.X, op=OP.add)
                dma_batch(kh_d, t)

            def qnorm(p, t):
                # Q path: full normalize (the per-query scale must multiply
                # inside the logits before exp -- it does not commute out).
                # The 1/sqrt(HD)=1/8 logit scale folds into the Sqrt scale
                # (the qk-norm scale vectors are ones): with p=32q and HD=64,
                # rs = 1/(8*32*rms_q) = 1/sqrt(1.0*ss + eps').
                praw = scr.tile([128, D], BF16, name="praw_b", tag="praw_b")
                nc.scalar.activation(praw[:], p[:], AF.Copy)
                sqv = scr.tile([128, D], BF16, name="sq_b", tag="sq_b")
                # gpsimd: DVE is the phase-B backlog engine and its queue
                # depth here decides how long the PE starves at the Q/V tail
                nc.gpsimd.tensor_tensor(out=sqv[:], in0=praw[:], in1=praw[:],
                                        op=OP.mult)
                ss = st_p.tile([128, H], F32, name="ss_b")
                nc.vector.tensor_reduce(
                    ss[:], sqv[:].rearrange("p (h k) -> p h k", h=H),
                    axis=mybir.AxisListType.X, op=OP.add)
                srt = st_p.tile([128, H], F32, name="srt_b")
                nc.scalar.activation(srt[:], ss[:], AF.Sqrt, bias=eps1k_t[:],
                                     scale=1.0)
                rs = st_p.tile([128, H], F32, name="rs_b")
                nc.vector.reciprocal_approx_fast(rs[:], srt[:])
                rsb = st_p.tile([128, H], BF16, name="rsb_b")
                nc.vector.tensor_copy(rsb[:], rs[:])
                if t % 4 == 0:
                    natg[0] = natp.tile([128, 4, D], BF16, name="nat_b",
                                        tag="nat_b")
                nat = natg[0][:, t % 4]
                rs_view = rsb[:].rearrange("p (h o) -> p h o", o=1).broadcast_to([128, H, HD])
                nc.vector.tensor_tensor(
                    out=nat.rearrange("p (h k) -> p h k", h=H),
                    in0=praw[:].rearrange("p (h k) -> p h k", h=H),
                    in1=rs_view, op=OP.mult)
                dma_batch(qh_d, t)

            # K loop first: kh_d streams out; the 6 big transposes issue once
            # kh_d is complete and overlap the V/Q loop.
            for t in range(NT_S):
                pk = proj(t, wk_sb)
                knorm(pk, t)
            for d in range(NT_D):
                nc.sync.dma_start_transpose(KT[d][:],
                                            kh_d[:, d * 128:(d + 1) * 128])

            # Q and V tiles interleaved: V's ACT evacuation is independent of
            # the Q-side DVE chain, so the PE keeps a runnable matmul stream
            # while Q's stats drain on DVE.
            def vtile(t):
                pv = proj(t, wv_sb)
                # ln1 row scale (host-precomputed rs_t, incl. the 1/32 fp8
                # compensation) folded into V's evacuation on ACT.
                pvv = pv[:].rearrange("p (h e k) -> p h e k", h=6, e=2)
                nc.scalar.activation(
                    v8view[:, t, :, 0:HD], pvv[:, :, 0],
                    AF.Copy, scale=rs_t[:, t:t + 1])
                nc.scalar.activation(
                    v16view[:, t, :, 0:HD], pvv[:, :, 1],
                    AF.Copy, scale=rs_t[:, t:t + 1])

            for t in range(NT_Q):
                pq = proj(t, wq_sb)
                qnorm(pq, t)
                # V interleave: vtile's ACT/PE work is independent of the
                # Q-side DVE chain, keeping the PE fed while it drains
                vtile(t)
            for d in range(NT_D):
                # scalar HWDGE ring: halves the sync queue's transpose backlog
                nc.scalar.dma_start_transpose(QTf[d][:],
                                              qh_d[:, d * 128:(d + 1) * 128])
            for d in range(NT_D):
                nc.scalar.dma_start(wo_sb[d][:], wo[d * 128:(d + 1) * 128, :])
            for t in range(NT_Q, NT_S):
                vtile(t)
            # Deferred: one Sqrt+reciprocal for ALL 16 K tiles' stats
            # (rsK = 1/(32*rms_k); 32 = fp8 weight pre-scale). Emitted last so
            # it never head-of-line-blocks the ACT FIFO mid-phase; phase C's
            # first exp is the only consumer.
            nc.scalar.activation(srtK_t[:], ssK[:], AF.Sqrt, bias=eps1k_t[:],
                                 scale=1.0 / HD)
            nc.vector.reciprocal_approx_fast(
                rsK[:].rearrange("p t h -> p (t h)"), srtK_t[:])
            nc.vector.tensor_scalar_mul(rsKc1[:], rsK[:], FEXP_C1)
            # zero-padded per-head Q^T tiles
            for d in range(NT_D):
                nc.vector.tensor_copy(QTz[d][0][0:64, :], QTf[d][0:64, :])
                nc.vector.tensor_copy(QTz[d][1][64:128, :], QTf[d][64:128, :])
        p_qtf.release()

        # =============== Phase C: attention ===============
        with ExitStack() as ctx:
            psL = ctx.enter_context(tc.tile_pool(name="c_psL", bufs=2, space="PSUM"))
            psO = ctx.enter_context(tc.tile_pool(name="c_psO", bufs=2, space="PSUM"))
            pp = ctx.enter_context(tc.tile_pool(name="c_p", bufs=6))
            oup = ctx.enter_context(tc.tile_pool(name="c_oU", bufs=4))
            dnp = ctx.enter_context(tc.tile_pool(name="c_dn", bufs=1))

            def divide_pair(hp, oU):
                # oT[64e:64e+64, hp*SQ+q] = oU[e][0:64, q] / denom_e[q]
                b_ps = psL.tile([128, SQ], F32, name="b_ps", tag="l_ps")
                for ofs, n in _chunks(SQ):
                    nc.tensor.matmul(b_ps[:, ofs:ofs + n], bc_pair[:],
                                     rT_pad[:, ofs:ofs + n], start=True, stop=True)
                    for e in range(2):
                        nc.vector.scalar_tensor_tensor(
                            oT[64 * e:64 * e + 64, hp * SQ + ofs:hp * SQ + ofs + n],
                            b_ps[64 * e:64 * e + 64, ofs:ofs + n], 1.0,
                            oU[e][0:HD, ofs:ofs + n], op0=OP.bypass, op1=OP.mult)

            def av_e1(hp, t, o_ps, p_rhs):
                # odd head: bf16 AV, one key-tile per matmul
                vofs = (t * 6 + hp) * VW
                for ofs, n in _chunks(SQ):
                    nc.tensor.matmul(
                        o_ps[1][:, ofs:ofs + n],
                        va16[:, vofs:vofs + 128],
                        p_rhs[:, ofs:ofs + n],
                        start=(t == 0), stop=(t == NT_S - 1))

            def av_e0(hp, tp, o_ps, p8p):
                # even head: fp8 DoubleRow AV, TWO key-tiles (2tp, 2tp+1)
                # contracted per matmul -- 2x PE throughput on this half.
                w = va8[:, 2 * tp:2 * tp + 2, hp]
                for ofs, n in _chunks(SQ):
                    nc.tensor.matmul(
                        o_ps[0][:, ofs:ofs + n], w,
                        p8p[:, :, ofs:ofs + n],
                        start=(tp == 0), stop=(tp == NT_S // 2 - 1),
                        perf_mode=mybir.MatmulPerfMode.DoubleRow)

            def pair_end(hp, o_ps):
                # evacuate accumulators (e0 on ACT, e1 on DVE), batch the
                # denominator recip. denoms sit at partitions 0 and 64 of
                # dn_p (rows 1..63 are a constant 1.0 so the batched
                # reciprocal stays finite there).
                oU = [oup.tile([VW, SQ], F32, name="oU", tag="oU")
                      for _ in range(2)]
                rf = dnp.tile([VW, SQ], F32, name="rf_pair", tag="rf")
                # chunked so the recip/divide chain starts on the first half
                # while the second half still copies -- the LAST pair's chain
                # gates phase D's first out-projection matmul.
                for ofs, n in _chunks(SQ):
                    nc.scalar.activation(oU[0][:, ofs:ofs + n],
                                         o_ps[0][0:VW, ofs:ofs + n], AF.Copy)
                    nc.vector.tensor_copy(oU[1][:, ofs:ofs + n],
                                          o_ps[1][0:VW, ofs:ofs + n])
                    for e in range(2):
                        nc.vector.tensor_copy(
                            dn_p[64 * e:64 * e + 1, ofs:ofs + n],
                            oU[e][HD:VW, ofs:ofs + n])
                    nc.vector.reciprocal_approx_fast(rf[:, ofs:ofs + n],
                                         dn_p[:, ofs:ofs + n])
                    nc.vector.tensor_copy(rT_pad[0:VW, ofs:ofs + n],
                                          rf[:, ofs:ofs + n])
                return (hp, oU)

            pending = None
            pending_ops = None
            for hp in range(H // 2):
                o_ps = [psO.tile([128, SQ], F32, name=f"o_ps{e}", tag="o_ps")
                        for e in range(2)]
                prev_e1 = None
                p8p = None
                for t in range(NT_S):
                    l_ps = [psL.tile([128, SQ], F32, name=f"l_ps{e}", tag="l_ps")
                            for e in range(2)]
                    ktile = KT[hp][:, t * 128:(t + 1) * 128]
                    for e in range(2):
                        for ofs, n in _chunks(SQ):
                            nc.tensor.matmul(
                                l_ps[e][:, ofs:ofs + n], ktile,
                                QTz[hp][e][:, ofs:ofs + n],
                                start=True, stop=True)
                    # e=0 (even head): ACT spline exp with the per-key rsK
                    # scale AP; output fp8 (exp bias shifts by -5ln2 so the
                    # max prob e^8 lands under e4m3's 240; the 1/32 cancels
                    # through the softmax divide). Pairs of t-tiles share one
                    # fp8 tile consumed by the DoubleRow AV.
                    # e=1 (odd head): DVE 16-bit Schraudolph fast-exp 3 of 4
                    # tiles (ACT was the attention wall at 164us solo).
                    if t % 2 == 0:
                        p8p = pp.tile([128, 2, SQ], mybir.dt.float8e4,
                                      name="p8p", tag="p8p")
                    idx = t * H + 2 * hp
                    nc.scalar.activation(
                        p8p[:, t % 2], l_ps[0][:], AF.Exp,
                        bias=mexp5_t[:],
                        scale=rsK[:].rearrange("p t h -> p (t h)")[
                            :, idx:idx + 1])
                    if t % 4 != 3:
                        p_i = pp.tile([128, SQ], mybir.dt.int16,
                                      name="p_i1", tag="p_i")
                        nc.vector.tensor_scalar(p_i[:], l_ps[1][:],
                                                rsKc1[:].rearrange(
                                                    "p t h -> p (t h)")[
                                                    :, idx + 1:idx + 2],
                                                FEXP_C2,
                                                op0=OP.mult, op1=OP.add)
                        p_e1 = p_i[:].bitcast(BF16)
                    else:
                        p_t = pp.tile([128, SQ], BF16, name="p_t1", tag="p_t")
                        nc.scalar.activation(
                            p_t[:], l_ps[1][:], AF.Exp,
                            scale=rsK[:].rearrange("p t h -> p (t h)")[
                                :, idx + 1:idx + 2])
                        p_e1 = p_t[:]
                    # Software pipeline: AV for tile t-1 (pair tp-1) issues
                    # after the logits for tile t, so the exp hides under the
                    # next tile's logits instead of stalling the in-order PE.
                    if prev_e1 is not None:
                        av_e1(hp, prev_e1[0], o_ps, prev_e1[1])
                    prev_e1 = (t, p_e1)
                    if t % 2 == 0 and t >= 2:
                        av_e0(hp, t // 2 - 1, o_ps, prev_p8p)
                    if t % 2 == 1:
                        prev_p8p = p8p
                    if t == 1 and pending_ops is not None:
                        pending = pair_end(*pending_ops)
                        pending_ops = None
                    if t == 4 and pending is not None:
                        divide_pair(*pending)
                        pending = None
                av_e1(hp, prev_e1[0], o_ps, prev_e1[1])
                av_e0(hp, NT_S // 2 - 1, o_ps, prev_p8p)
                # Pair-end evacuation + denominator recip are DEFERRED into
                # the next pair's t==1 slot (like the divide at t==4): a 3us
                # ACT/DVE burst at the boundary delayed the next pair's exps
                # (l_ps ring) and cold-clocked the PE for ~7us per boundary.
                pending_ops = (hp, o_ps)
            pending = pair_end(*pending_ops)
            divide_pair(*pending)
        p_att.release()
        p_xT.release()

        # =============== Phase D: out-proj^T + residual + ln2 ===============
        p_x2 = tc.alloc_tile_pool(name="p_x2", bufs=1)
        x2T = [ptile(p_x2, [128, SQ], F32, name=f"x2T{d}") for d in range(NT_D)]
        x2h = [ptile(p_x2, [128, 2, SQ], mybir.dt.float8e4, name=f"x2h{j}")
               for j in range(NT_D // 2)]
        p_lt2 = tc.alloc_tile_pool(name="p_lt2", bufs=1)
        lt2 = [ptile(p_lt2, [128, SQ], BF16, name=f"lt2_{d}") for d in range(NT_D)]
        for d in range(NT_D):
            nc.sync.dma_start(lt2[d][:], latb[d * 128:(d + 1) * 128, :])
        p_ew = tc.alloc_tile_pool(name="p_ew", bufs=1)
        wi_sb = [ptile(p_ew, [128, 2, MLP], mybir.dt.float8e4,
                       name=f"wi_sb{j}") for j in range(NT_D // 2)]
        wom_sb = [ptile(p_ew, [128, D], BF16, name=f"wom_sb{m}") for m in range(NT_M)]
        for j in range(NT_D // 2):
            nc.sync.dma_start(wi_sb[j][:],
                              wi[2 * j * 128:(2 * j + 2) * 128, :].rearrange(
                                  "(e p) s -> p e s", e=2))
        for m in range(NT_M):
            nc.sync.dma_start(wom_sb[m][:], wom[m * 128:(m + 1) * 128, :])
        with ExitStack() as ctx:
            psD = ctx.enter_context(tc.tile_pool(name="d_ps", bufs=2, space="PSUM"))
            sqp = ctx.enter_context(tc.tile_pool(name="d_sq", bufs=2))
            st_p = ctx.enter_context(tc.tile_pool(name="d_stats", bufs=2))
            srt2 = st_p.tile([1, SQ], F32, name="srt2", bufs=1)
            r2 = st_p.tile([1, SQ], F32, name="r2", bufs=1)
            # 256-wide chunks (4 of them) so each chunk's ln2 stats/recip
            # chain hides under the next chunk's projection matmuls.
            for ofs, n in _chunks(SQ, 256):
                x2sq = [sqp.tile([128, 256], BF16, name=f"x2sq{d}",
                                 tag=f"x2sq{d}") for d in range(NT_D)]
                for dc in range(NT_D):
                    xp = psD.tile([128, 256], F32, name="xp_d", bufs=4)
                    for dt in range(NT_D):
                        nc.tensor.matmul(
                            xp[:, 0:n],
                            wo_sb[dt][:, dc * 128:(dc + 1) * 128],
                            oT[:, dt * SQ + ofs:dt * SQ + ofs + n],
                            start=(dt == 0), stop=(dt == NT_D - 1))
                    nc.vector.tensor_tensor(
                        out=x2T[dc][:, ofs:ofs + n], in0=xp[:, 0:n],
                        in1=lt2[dc][:, ofs:ofs + n], op=OP.add)
                    nc.scalar.activation(x2sq[dc][:, 0:n],
                                         x2T[dc][:, ofs:ofs + n], AF.Square)
                sp2 = psD.tile([1, 256], F32, name="sp2_d")
                for dc in range(NT_D):
                    nc.tensor.matmul(sp2[:, 0:n], ones_col[:],
                                     x2sq[dc][:, 0:n],
                                     start=(dc == 0), stop=(dc == NT_D - 1))
                nc.scalar.activation(srt2[:, ofs:ofs + n], sp2[:, 0:n],
                                     AF.Sqrt, bias=eps_t[0:1, :], scale=1.0 / D)
                nc.vector.reciprocal_approx_fast(r2[:, ofs:ofs + n], srt2[:, ofs:ofs + n])
                nc.vector.tensor_copy(rs2_pad[0:1, ofs:ofs + n],
                                      r2[:, ofs:ofs + n])
                # r2 broadcast to 128 partitions once per chunk (it is
                # dc-independent), then 6 DVE multiplies read it from PSUM.
                b2 = psD.tile([128, 256], F32, name="b2_d")
                nc.tensor.matmul(b2[:, 0:n], bc_row0[:],
                                 rs2_pad[:, ofs:ofs + n], start=True, stop=True)
                for dc in range(NT_D):
                    nc.vector.tensor_tensor(
                        out=x2h[dc // 2][:, dc % 2, ofs:ofs + n],
                        in0=x2T[dc][:, ofs:ofs + n],
                        in1=b2[:, 0:n], op=OP.mult)

        # =============== Phase E: MLP (mlp2 transposed) ===============
        p_hT = tc.alloc_tile_pool(name="p_hT", bufs=1)
        hT = ptile(p_hT, [128, NT_M * SQ], BF16, name="hT")
        with ExitStack() as ctx:
            ps = ctx.enter_context(tc.tile_pool(name="e_ps", bufs=1, space="PSUM"))
            iop = ctx.enter_context(tc.tile_pool(name="e_io", bufs=3))

            for m in range(NT_M):
                p = ps.tile([128, SQ], F32, name="p_mlp1", bufs=2)
                for j in range(NT_D // 2):
                    for ofs, n in _chunks(SQ):
                        nc.tensor.matmul(
                            p[:, ofs:ofs + n],
                            wi_sb[j][:, :, m * 128:(m + 1) * 128],
                            x2h[j][:, :, ofs:ofs + n],
                            start=(j == 0), stop=(j == NT_D // 2 - 1),
                            perf_mode=mybir.MatmulPerfMode.DoubleRow)
                # 1/32 undoes the fp8 wi pre-scale, inside gelu's pre-affine
                if not sim_compat:
                    nc.scalar.activation(hT[:, m * SQ:(m + 1) * SQ], p[:],
                                         AF.Gelu_apprx_tanh, scale=1.0 / 32.0)
                else:
                    ga = iop.tile([128, SQ], F32, name="g_a", bufs=1, tag="g_a")
                    gb = iop.tile([128, SQ], F32, name="g_b", bufs=1, tag="g_b")
                    p32 = iop.tile([128, SQ], F32, name="p32", bufs=1, tag="p32")
                    nc.vector.tensor_scalar_mul(p32[:], p[:], 1.0 / 32.0)
                    p = p32
                    nc.vector.tensor_tensor(out=ga[:], in0=p[:], in1=p[:], op=OP.mult)
                    nc.vector.tensor_scalar(gb[:], ga[:], 0.044715, 1.0,
                                            op0=OP.mult, op1=OP.add)
                    nc.vector.tensor_tensor(out=ga[:], in0=gb[:], in1=p[:], op=OP.mult)
                    nc.scalar.activation(gb[:], ga[:], AF.Tanh, scale=0.7978845608028654)
                    nc.vector.scalar_tensor_tensor(ga[:], gb[:], 1.0, p[:],
                                                   op0=OP.add, op1=OP.mult)
                    nc.vector.tensor_scalar_mul(hT[:, m * SQ:(m + 1) * SQ], ga[:], 0.5)

            for dc in range(NT_D):
                for ofs, n in _chunks(SQ):
                    yp = ps.tile([128, 512], F32, name="p_mlp2", bufs=3)
                    for m in range(NT_M):
                        nc.tensor.matmul(
                            yp[:, 0:n],
                            wom_sb[m][:, dc * 128:(dc + 1) * 128],
                            hT[:, m * SQ + ofs:m * SQ + ofs + n],
                            start=(m == 0), stop=(m == NT_M - 1))
                    ot = iop.tile([128, 512], F32, name="ot_e")
                    nc.vector.tensor_tensor(out=ot[:, 0:n], in0=yp[:, 0:n],
                                            in1=x2T[dc][:, ofs:ofs + n], op=OP.add)
                    nc.sync.dma_start(
                        out[dc * 128:(dc + 1) * 128, ofs:ofs + n], ot[:, 0:n])
        p_hT.release()
        p_ew.release()
        p_lt2.release()
        p_x2.release()
        p_wo.release()
        p_oT.release()

    nc.compile()
    return nc


def make_in_maps(latents, ln1_scale, wq, wk, wv, q_norm_scale, k_norm_scale,
                 wo_attn, ln2_scale, wi, wo_mlp):
    import ml_dtypes
    bf = ml_dtypes.bfloat16
    f8 = ml_dtypes.float8_e4m3fn
    ln1 = np.asarray(ln1_scale, np.float64)[:, None]
    wq2 = (32.0 * ln1 * np.asarray(wq, np.float64).reshape(D, D)).astype(f8)
    wk2 = (32.0 * ln1 * np.asarray(wk, np.float64).reshape(D, D)).astype(f8)
    wv2 = (32.0 * ln1 * np.asarray(wv, np.float64).reshape(D, D)).astype(f8)
    wo2 = np.asarray(wo_attn, np.float32).reshape(D, D).astype(bf)
    wi2 = (32.0 * np.asarray(ln2_scale, np.float64)[:, None]
           * np.asarray(wi, np.float64)).astype(f8)
    wom2 = np.asarray(wo_mlp, np.float32).astype(bf)
    assert np.allclose(np.asarray(q_norm_scale), 1.0) and \
        np.allclose(np.asarray(k_norm_scale), 1.0), \
        "qk-norm scales folded assuming ones"
    lat_np = np.asarray(latents, np.float32)
    in_maps = []
    for c in range(8):
        b, half = c // 2, c % 2
        lm = lat_np[b]
        lat_rot = np.concatenate([lm[half * SQ:(half + 1) * SQ],
                                  lm[(1 - half) * SQ:(2 - half) * SQ]], axis=0)
        lat8 = np.ascontiguousarray(lat_rot.T.astype(f8))
        latb = np.ascontiguousarray(lat_rot[0:SQ].T.astype(bf))
        rs = 1.0 / (32.0 * np.sqrt((lat_rot.astype(np.float64) ** 2).mean(-1)
                                   + 1e-6))
        rst = np.ascontiguousarray(
            rs.reshape(NT_S, 128).T.astype(np.float32))
        in_maps.append(dict(lat8=lat8, latb=latb, rst=rst, wq=wq2, wk=wk2,
                            wv=wv2, wo=wo2, wi=wi2, wom=wom2))
    return in_maps


_NC_CACHE = None


def kernel(**inputs):
    global _NC_CACHE
    if _NC_CACHE is None:
        _NC_CACHE = build_nc()
    nc = _NC_CACHE
    in_maps = make_in_maps(**inputs)
    res = run_bass_kernel_spmd(nc, in_maps, list(range(8)))
    y = np.empty((B, S, D), np.float32)
    for c in range(8):
        b, half = c // 2, c % 2
        y[b, half * SQ:(half + 1) * SQ] = np.asarray(res.results[c]["out"]).T
    return y


if __name__ == "__main__":
    import reference
    inputs = {k: np.asarray(v) for k, v in reference.setup_inputs().items()}
    y = kernel(**inputs)
    exp = np.asarray(reference.reference(**reference.setup_inputs()))
    err = np.abs(y - exp).max() / np.abs(exp).max()
    print("Relative error:", err)

